# revision 1
# baseline (speedup 1.0000x reference)
"""Trainium2 Bass kernel for the gnn_message_passing problem.

Math reduction: the reference builds a [8192,8192] zero-diagonal adjacency
W_full from per-node Linear(8191,1) weights, forms state = [x | zeros] and
returns (state @ W_full.T + bias)[:, 7168:][:, ::-1].

Because state is zero outside its first 1024 columns, and only output nodes
7168..8191 are read, the whole computation collapses to

    out[b, k] = sum_c x[b, c] * weights[8191-k, c] + bias[8191-k]

i.e. a [32,1024] x [1024,1024]^T matmul + bias (for rows n >= 7168 and
cols c < 1024 we always have c < n, so W_full[n, c] == weights[n, c]).

Distribution: shard the 1024 output features row-wise across 8 cores
(128 each, tensor parallel); every core holds the replicated x. No
collectives — the host concatenates the 8 output slices.

Per-core Bass kernel: out_slice[k', b] = sum_c W_slice[k', c] * xT[c, b]
computed as 8 PSUM-accumulated matmuls over the contraction dim (1024),
bias added on the scalar engine (per-partition bias), single DMA per
operand with host-side packing so every DMA is contiguous per partition.
"""

import numpy as np

import concourse.bacc as bacc
import concourse.bass as bass
import concourse.mybir as mybir
from concourse.bass_utils import run_bass_kernel_spmd
from concourse.tile import TileContext

NODES = 8192
IN_F = 1024
OUT_F = 1024
B = 32
N_CORES = 8
KPC = OUT_F // N_CORES  # output features per core: 128
NCHUNK = IN_F // 128    # contraction chunks: 8

F32 = mybir.dt.float32

_NC = None
LAST_RESULT = None  # BassKernelResults of the most recent run (for profiling)


def _build_nc():
    nc = bacc.Bacc(None, target_bir_lowering=False)

    # Per-core inputs, pre-packed on host so partition dim is contiguous:
    #   wt[p, n*KPC + k'] = W_eff[core*KPC + k', n*128 + p]
    #   xt[p, n*B   + b ] = x[b, n*128 + p]
    wt = nc.dram_tensor("wt", [128, NCHUNK * KPC], F32, kind="ExternalInput")
    xt = nc.dram_tensor("xt", [128, NCHUNK * B], F32, kind="ExternalInput")
    bi = nc.dram_tensor("bi", [KPC, 1], F32, kind="ExternalInput")
    out = nc.dram_tensor("out", [KPC, B], F32, kind="ExternalOutput")

    with TileContext(nc) as tc:
        with (
            tc.tile_pool(name="sbuf", bufs=1) as pool,
            tc.tile_pool(name="psum", bufs=1, space=bass.MemorySpace.PSUM) as psum_pool,
        ):
            wt_t = pool.tile([128, NCHUNK * KPC], F32)
            xt_t = pool.tile([128, NCHUNK * B], F32)
            b_t = pool.tile([KPC, 1], F32)
            o_t = pool.tile([KPC, B], F32)
            ps = psum_pool.tile([KPC, B], F32)

            nc.sync.dma_start(wt_t[:], wt[:])
            nc.sync.dma_start(xt_t[:], xt[:])
            nc.sync.dma_start(b_t[:], bi[:])

            for n in range(NCHUNK):
                nc.tensor.matmul(
                    ps[:],
                    wt_t[:, n * KPC : (n + 1) * KPC],  # lhsT [c=128, k'=128]
                    xt_t[:, n * B : (n + 1) * B],      # rhs  [c=128, b=32]
                    start=(n == 0),
                    stop=(n == NCHUNK - 1),
                )

            nc.scalar.activation(
                o_t[:], ps[:], mybir.ActivationFunctionType.Identity, bias=b_t[:]
            )
            nc.sync.dma_start(out[:], o_t[:])

    nc.finalize()
    return nc


def kernel(x: np.ndarray, weights: np.ndarray, bias: np.ndarray) -> np.ndarray:
    global _NC, LAST_RESULT
    if _NC is None:
        _NC = _build_nc()

    x = np.ascontiguousarray(np.asarray(x, dtype=np.float32))
    weights = np.asarray(weights, dtype=np.float32)
    bias = np.asarray(bias, dtype=np.float32)

    # Effective dense weight block and bias (see module docstring).
    w_eff = weights[NODES - OUT_F :, :IN_F][::-1]  # [1024 (k), 1024 (c)]
    b_eff = bias[NODES - OUT_F :][::-1]            # [1024]

    # Pack per-core operands. w_eff[(i,k'),(n,p)] -> wt[i][p, (n,k')]
    wt_all = w_eff.reshape(N_CORES, KPC, NCHUNK, 128).transpose(0, 3, 2, 1)
    wt_all = np.ascontiguousarray(wt_all.reshape(N_CORES, 128, NCHUNK * KPC))
    # x[b, (n,p)] -> xt[p, (n,b)], replicated
    xt = np.ascontiguousarray(
        x.reshape(B, NCHUNK, 128).transpose(2, 1, 0).reshape(128, NCHUNK * B)
    )
    b_all = np.ascontiguousarray(b_eff.reshape(N_CORES, KPC, 1))

    in_maps = [
        {"wt": wt_all[i], "xt": xt, "bi": b_all[i]} for i in range(N_CORES)
    ]
    LAST_RESULT = run_bass_kernel_spmd(_NC, in_maps, list(range(N_CORES)))

    # Gather: core i returns out[k', b] for k = i*KPC + k'.
    out_t = np.concatenate([r["out"] for r in LAST_RESULT.results], axis=0)
    return np.ascontiguousarray(out_t.T)



# revision 2
# speedup vs baseline: 1.2527x; 1.2527x over previous
"""Trainium2 Bass kernel for the gnn_message_passing problem.

Math reduction: the reference builds a [8192,8192] zero-diagonal adjacency
W_full from per-node Linear(8191,1) weights, forms state = [x | zeros] and
returns (state @ W_full.T + bias)[:, 7168:][:, ::-1].

Because state is zero outside its first 1024 columns, and only output nodes
7168..8191 are read, the whole computation collapses to

    out[b, k] = sum_c x[b, c] * weights[8191-k, c] + bias[8191-k]

i.e. a [32,1024] x [1024,1024]^T matmul + bias (for rows n >= 7168 and
cols c < 1024 we always have c < n, so W_full[n, c] == weights[n, c]).

Distribution: shard the 1024 output features row-wise across 8 cores
(128 each, tensor parallel); every core holds the replicated x. No
collectives — the host concatenates the 8 output slices.

Per-core Bass kernel (latency-optimized, the problem is tiny):
  - weights and x are cast to bf16 on the host (rel err ~1e-3, far inside
    the 2e-2 gate): halves HBM traffic and doubles PE throughput.
  - x + bias are DMAed on the ACT HWDGE queue while the weight tile
    streams on the SP queue in two chunks, so the first 4 PSUM-accumulated
    matmuls overlap the tail of the weight DMA.
  - bias add on the vector engine (tensor_scalar_add) instead of the
    scalar activation path — avoids a 1.3us ACT_TABLE_LOAD.
"""

import numpy as np
import ml_dtypes

import concourse.bacc as bacc
import concourse.bass as bass
import concourse.mybir as mybir
from concourse.bass_utils import run_bass_kernel_spmd
from concourse.tile import TileContext

NODES = 8192
IN_F = 1024
OUT_F = 1024
B = 32
N_CORES = 8
KPC = OUT_F // N_CORES  # output features per core: 128
NCHUNK = IN_F // 128    # contraction chunks: 8

F32 = mybir.dt.float32
BF16 = mybir.dt.bfloat16

_NC = None
LAST_RESULT = None  # BassKernelResults of the most recent run (for profiling)


def _build_nc():
    nc = bacc.Bacc(None, target_bir_lowering=False)

    # Per-core inputs, pre-packed on host so partition dim is contiguous:
    #   wt[p, n*KPC + k'] = W_eff[core*KPC + k', n*128 + p]   (bf16)
    #   xt[p, n*B   + b ] = x[b, n*128 + p]                   (bf16)
    wt = nc.dram_tensor("wt", [128, NCHUNK * KPC], BF16, kind="ExternalInput")
    xt = nc.dram_tensor("xt", [128, NCHUNK * B], BF16, kind="ExternalInput")
    bi = nc.dram_tensor("bi", [KPC, 1], F32, kind="ExternalInput")
    out = nc.dram_tensor("out", [KPC, B], F32, kind="ExternalOutput")

    with TileContext(nc) as tc:
        with (
            tc.tile_pool(name="sbuf", bufs=1) as pool,
            tc.tile_pool(name="psum", bufs=1, space=bass.MemorySpace.PSUM) as psum_pool,
        ):
            wt_t = pool.tile([128, NCHUNK * KPC], BF16)
            xt_t = pool.tile([128, NCHUNK * B], BF16)
            b_t = pool.tile([KPC, 1], F32)
            o_t = pool.tile([KPC, B], F32)
            ps = psum_pool.tile([KPC, B], F32)

            # Small operands on the ACT HWDGE queue; weight stream on SP in
            # two chunks so matmuls can start when the first half lands.
            nc.scalar.dma_start(xt_t[:], xt[:])
            nc.scalar.dma_start(b_t[:], bi[:])
            half = NCHUNK * KPC // 2
            nc.sync.dma_start(wt_t[:, :half], wt[:, :half])
            nc.sync.dma_start(wt_t[:, half:], wt[:, half:])

            for n in range(NCHUNK):
                nc.tensor.matmul(
                    ps[:],
                    wt_t[:, n * KPC : (n + 1) * KPC],  # lhsT [c=128, k'=128]
                    xt_t[:, n * B : (n + 1) * B],      # rhs  [c=128, b=32]
                    start=(n == 0),
                    stop=(n == NCHUNK - 1),
                )

            nc.vector.tensor_scalar_add(o_t[:], ps[:], b_t[:])
            nc.sync.dma_start(out[:], o_t[:])

    nc.finalize()
    return nc


def kernel(x: np.ndarray, weights: np.ndarray, bias: np.ndarray) -> np.ndarray:
    global _NC, LAST_RESULT
    if _NC is None:
        _NC = _build_nc()

    x = np.ascontiguousarray(np.asarray(x, dtype=np.float32))
    weights = np.asarray(weights, dtype=np.float32)
    bias = np.asarray(bias, dtype=np.float32)

    # Effective dense weight block and bias (see module docstring).
    w_eff = weights[NODES - OUT_F :, :IN_F][::-1]  # [1024 (k), 1024 (c)]
    b_eff = bias[NODES - OUT_F :][::-1]            # [1024]

    # Pack per-core operands. w_eff[(i,k'),(n,p)] -> wt[i][p, (n,k')]
    wt_all = w_eff.reshape(N_CORES, KPC, NCHUNK, 128).transpose(0, 3, 2, 1)
    wt_all = np.ascontiguousarray(
        wt_all.reshape(N_CORES, 128, NCHUNK * KPC).astype(ml_dtypes.bfloat16)
    )
    # x[b, (n,p)] -> xt[p, (n,b)], replicated
    xt = np.ascontiguousarray(
        x.reshape(B, NCHUNK, 128).transpose(2, 1, 0).reshape(128, NCHUNK * B)
        .astype(ml_dtypes.bfloat16)
    )
    b_all = np.ascontiguousarray(b_eff.reshape(N_CORES, KPC, 1))

    in_maps = [
        {"wt": wt_all[i], "xt": xt, "bi": b_all[i]} for i in range(N_CORES)
    ]
    LAST_RESULT = run_bass_kernel_spmd(_NC, in_maps, list(range(N_CORES)))

    # Gather: core i returns out[k', b] for k = i*KPC + k'.
    out_t = np.concatenate([r["out"] for r in LAST_RESULT.results], axis=0)
    return np.ascontiguousarray(out_t.T)


# revision 4
# speedup vs baseline: 1.7851x; 1.4250x over previous
"""Trainium2 Bass kernel for the gnn_message_passing problem.

Math reduction: the reference builds a [8192,8192] zero-diagonal adjacency
W_full from per-node Linear(8191,1) weights, forms state = [x | zeros] and
returns (state @ W_full.T + bias)[:, 7168:][:, ::-1].

Because state is zero outside its first 1024 columns, and only output nodes
7168..8191 are read, the whole computation collapses to

    out[b, k] = sum_c x[b, c] * weights[8191-k, c] + bias[8191-k]

i.e. a [32,1024] x [1024,1024]^T matmul + bias (for rows n >= 7168 and
cols c < 1024 we always have c < n, so W_full[n, c] == weights[n, c]).

Distribution: shard the 1024 output features row-wise across 8 cores
(128 each, tensor parallel); every core holds the replicated x. No
collectives — the host concatenates the 8 output slices.

Per-core Bass kernel (latency-optimized, the problem is tiny):
  - weights and x are cast to bf16 on the host (rel err ~1e-3, far inside
    the 2e-2 gate): halves HBM traffic and doubles PE throughput.
  - x + bias are DMAed on the ACT HWDGE queue while the weight tile
    streams on the SP queue in two chunks, so the first 4 PSUM-accumulated
    matmuls overlap the tail of the weight DMA.
  - bias add on the vector engine (tensor_scalar_add) instead of the
    scalar activation path — avoids a 1.3us ACT_TABLE_LOAD.
"""

import numpy as np
import ml_dtypes

import concourse.bacc as bacc
import concourse.bass as bass
import concourse.mybir as mybir
from concourse.bass_utils import run_bass_kernel_spmd
from concourse.tile import TileContext

NODES = 8192
IN_F = 1024
OUT_F = 1024
B = 32
N_CORES = 8
KPC = OUT_F // N_CORES  # output features per core: 128
NCHUNK = IN_F // 128    # contraction chunks: 8

F32 = mybir.dt.float32
BF16 = mybir.dt.bfloat16

_NC = None
LAST_RESULT = None  # BassKernelResults of the most recent run (for profiling)


def _build_nc():
    nc = bacc.Bacc(None, target_bir_lowering=False)

    # Per-core inputs, pre-packed on host so partition dim is contiguous:
    #   wt[p, n*KPC + k'] = W_eff[core*KPC + k', n*128 + p]   (bf16)
    #   xt[p, n*B   + b ] = x[b, n*128 + p]                   (bf16)
    wt = nc.dram_tensor("wt", [128, NCHUNK * KPC], BF16, kind="ExternalInput")
    xt = nc.dram_tensor("xt", [128, NCHUNK * B], BF16, kind="ExternalInput")
    bi = nc.dram_tensor("bi", [KPC, 1], F32, kind="ExternalInput")
    out = nc.dram_tensor("out", [KPC, B], F32, kind="ExternalOutput")

    with TileContext(nc) as tc:
        with (
            tc.tile_pool(name="sbuf", bufs=1) as pool,
            tc.tile_pool(name="psum", bufs=1, space=bass.MemorySpace.PSUM) as psum_pool,
        ):
            wt_t = pool.tile([128, NCHUNK * KPC], BF16)
            xt_t = pool.tile([128, NCHUNK * B], BF16)
            b_t = pool.tile([KPC, 1], F32)
            o_t = pool.tile([KPC, B], F32)
            ps = psum_pool.tile([KPC, B], F32)

            # Split the weight stream across both HWDGE queues (SP + ACT) so
            # the two halves issue and drain in parallel; small operands ride
            # second on each queue.
            half = NCHUNK * KPC // 2
            nc.sync.dma_start(wt_t[:, :half], wt[:, :half])
            nc.scalar.dma_start(wt_t[:, half:], wt[:, half:])
            nc.sync.dma_start(xt_t[:], xt[:])
            nc.scalar.dma_start(b_t[:], bi[:])

            for n in range(NCHUNK):
                nc.tensor.matmul(
                    ps[:],
                    wt_t[:, n * KPC : (n + 1) * KPC],  # lhsT [c=128, k'=128]
                    xt_t[:, n * B : (n + 1) * B],      # rhs  [c=128, b=32]
                    start=(n == 0),
                    stop=(n == NCHUNK - 1),
                )

            nc.vector.tensor_scalar_add(o_t[:], ps[:], b_t[:])
            nc.sync.dma_start(out[:], o_t[:])

    # Drop the framework's const-tile memsets ([128,1] constants 0.0/1.0/...)
    # — nothing in this kernel reads them, and they are the only datapath
    # instructions ahead of the DMA issue, so they both waste GpSimd work and
    # drag the profiled start ~3.5us before any real work.
    blk = nc.m.functions[0].blocks[0]
    for inst in [i for i in blk.instructions if isinstance(i, mybir.InstMemset)]:
        blk.instructions.remove(inst)

    nc.finalize()
    return nc


def kernel(x: np.ndarray, weights: np.ndarray, bias: np.ndarray) -> np.ndarray:
    global _NC, LAST_RESULT
    if _NC is None:
        _NC = _build_nc()

    x = np.ascontiguousarray(np.asarray(x, dtype=np.float32))
    weights = np.asarray(weights, dtype=np.float32)
    bias = np.asarray(bias, dtype=np.float32)

    # Effective dense weight block and bias (see module docstring).
    w_eff = weights[NODES - OUT_F :, :IN_F][::-1]  # [1024 (k), 1024 (c)]
    b_eff = bias[NODES - OUT_F :][::-1]            # [1024]

    # Pack per-core operands. w_eff[(i,k'),(n,p)] -> wt[i][p, (n,k')]
    wt_all = w_eff.reshape(N_CORES, KPC, NCHUNK, 128).transpose(0, 3, 2, 1)
    wt_all = np.ascontiguousarray(
        wt_all.reshape(N_CORES, 128, NCHUNK * KPC).astype(ml_dtypes.bfloat16)
    )
    # x[b, (n,p)] -> xt[p, (n,b)], replicated
    xt = np.ascontiguousarray(
        x.reshape(B, NCHUNK, 128).transpose(2, 1, 0).reshape(128, NCHUNK * B)
        .astype(ml_dtypes.bfloat16)
    )
    b_all = np.ascontiguousarray(b_eff.reshape(N_CORES, KPC, 1))

    in_maps = [
        {"wt": wt_all[i], "xt": xt, "bi": b_all[i]} for i in range(N_CORES)
    ]
    LAST_RESULT = run_bass_kernel_spmd(_NC, in_maps, list(range(N_CORES)))

    # Gather: core i returns out[k', b] for k = i*KPC + k'.
    out_t = np.concatenate([r["out"] for r in LAST_RESULT.results], axis=0)
    return np.ascontiguousarray(out_t.T)


# revision 5
# speedup vs baseline: 1.8043x; 1.0108x over previous
"""Trainium2 Bass kernel for the gnn_message_passing problem.

Math reduction: the reference builds a [8192,8192] zero-diagonal adjacency
W_full from per-node Linear(8191,1) weights, forms state = [x | zeros] and
returns (state @ W_full.T + bias)[:, 7168:][:, ::-1].

Because state is zero outside its first 1024 columns, and only output nodes
7168..8191 are read, the whole computation collapses to

    out[b, k] = sum_c x[b, c] * weights[8191-k, c] + bias[8191-k]

i.e. a [32,1024] x [1024,1024]^T matmul + bias (for rows n >= 7168 and
cols c < 1024 we always have c < n, so W_full[n, c] == weights[n, c]).

Distribution: shard the 1024 output features row-wise across 8 cores
(128 each, tensor parallel); every core holds the replicated x. No
collectives — the host concatenates the 8 output slices.

Per-core Bass kernel (latency-optimized, the problem is tiny):
  - weights and x are cast to bf16 on the host (rel err ~1e-3, far inside
    the 2e-2 gate): halves HBM traffic and doubles PE throughput.
  - x + bias are DMAed on the ACT HWDGE queue while the weight tile
    streams on the SP queue in two chunks, so the first 4 PSUM-accumulated
    matmuls overlap the tail of the weight DMA.
  - bias add on the vector engine (tensor_scalar_add) instead of the
    scalar activation path — avoids a 1.3us ACT_TABLE_LOAD.
"""

import numpy as np
import ml_dtypes

import concourse.bacc as bacc
import concourse.bass as bass
import concourse.mybir as mybir
from concourse.bass_utils import run_bass_kernel_spmd
from concourse.tile import TileContext

NODES = 8192
IN_F = 1024
OUT_F = 1024
B = 32
N_CORES = 8
KPC = OUT_F // N_CORES  # output features per core: 128
NCHUNK = IN_F // 128    # contraction chunks: 8

F32 = mybir.dt.float32
BF16 = mybir.dt.bfloat16

_NC = None
LAST_RESULT = None  # BassKernelResults of the most recent run (for profiling)


def _build_nc():
    nc = bacc.Bacc(None, target_bir_lowering=False)

    # Per-core inputs, pre-packed on host so partition dim is contiguous:
    #   wt[p, n*KPC + k'] = W_eff[core*KPC + k', n*128 + p]   (bf16)
    #   xt[p, n*B   + b ] = x[b, n*128 + p]                   (bf16)
    wt = nc.dram_tensor("wt", [128, NCHUNK * KPC], BF16, kind="ExternalInput")
    xt = nc.dram_tensor("xt", [128, NCHUNK * B], BF16, kind="ExternalInput")
    bi = nc.dram_tensor("bi", [KPC, 1], F32, kind="ExternalInput")
    out = nc.dram_tensor("out", [KPC, B], F32, kind="ExternalOutput")

    with TileContext(nc) as tc:
        with (
            tc.tile_pool(name="sbuf", bufs=1) as pool,
            tc.tile_pool(name="psum", bufs=1, space=bass.MemorySpace.PSUM) as psum_pool,
        ):
            wt_t = pool.tile([128, NCHUNK * KPC], BF16)
            xt_t = pool.tile([128, NCHUNK * B], BF16)
            b_t = pool.tile([KPC, 1], F32)
            o_t = pool.tile([KPC, B], F32)
            ps = psum_pool.tile([KPC, B], F32)

            # xt (small) first so it lands before the weight block: the PE
            # chain fires the moment the weight DMA completes, with no stall
            # between LDWEIGHTS and the first MATMUL. bias rides the ACT queue.
            nc.sync.dma_start(xt_t[:], xt[:])
            nc.sync.dma_start(wt_t[:], wt[:])
            nc.scalar.dma_start(b_t[:], bi[:])

            for n in range(NCHUNK):
                nc.tensor.matmul(
                    ps[:],
                    wt_t[:, n * KPC : (n + 1) * KPC],  # lhsT [c=128, k'=128]
                    xt_t[:, n * B : (n + 1) * B],      # rhs  [c=128, b=32]
                    start=(n == 0),
                    stop=(n == NCHUNK - 1),
                )

            nc.vector.tensor_scalar_add(o_t[:], ps[:], b_t[:])
            nc.sync.dma_start(out[:], o_t[:])

    # Drop the framework's const-tile memsets ([128,1] constants 0.0/1.0/...)
    # — nothing in this kernel reads them, and they are the only datapath
    # instructions ahead of the DMA issue, so they both waste GpSimd work and
    # drag the profiled start ~3.5us before any real work.
    blk = nc.m.functions[0].blocks[0]
    for inst in [i for i in blk.instructions if isinstance(i, mybir.InstMemset)]:
        blk.instructions.remove(inst)

    nc.finalize()
    return nc


def kernel(x: np.ndarray, weights: np.ndarray, bias: np.ndarray) -> np.ndarray:
    global _NC, LAST_RESULT
    if _NC is None:
        _NC = _build_nc()

    x = np.ascontiguousarray(np.asarray(x, dtype=np.float32))
    weights = np.asarray(weights, dtype=np.float32)
    bias = np.asarray(bias, dtype=np.float32)

    # Effective dense weight block and bias (see module docstring).
    w_eff = weights[NODES - OUT_F :, :IN_F][::-1]  # [1024 (k), 1024 (c)]
    b_eff = bias[NODES - OUT_F :][::-1]            # [1024]

    # Pack per-core operands. w_eff[(i,k'),(n,p)] -> wt[i][p, (n,k')]
    wt_all = w_eff.reshape(N_CORES, KPC, NCHUNK, 128).transpose(0, 3, 2, 1)
    wt_all = np.ascontiguousarray(
        wt_all.reshape(N_CORES, 128, NCHUNK * KPC).astype(ml_dtypes.bfloat16)
    )
    # x[b, (n,p)] -> xt[p, (n,b)], replicated
    xt = np.ascontiguousarray(
        x.reshape(B, NCHUNK, 128).transpose(2, 1, 0).reshape(128, NCHUNK * B)
        .astype(ml_dtypes.bfloat16)
    )
    b_all = np.ascontiguousarray(b_eff.reshape(N_CORES, KPC, 1))

    in_maps = [
        {"wt": wt_all[i], "xt": xt, "bi": b_all[i]} for i in range(N_CORES)
    ]
    LAST_RESULT = run_bass_kernel_spmd(_NC, in_maps, list(range(N_CORES)))

    # Gather: core i returns out[k', b] for k = i*KPC + k'.
    out_t = np.concatenate([r["out"] for r in LAST_RESULT.results], axis=0)
    return np.ascontiguousarray(out_t.T)
